# revision 7
# baseline (speedup 1.0000x reference)
"""ChromaLoss (mean CIEDE2000) on 8 Trainium2 NeuronCores.

Self-contained: kernel(img1, img2) -> np.float32 scalar (full output).
Data-parallel: each core takes 2 of the 16 image pairs; per-core partial
sums ([128, 8] fp32) are reduced on host.

Trig-free CIEDE2000 (validated vs reference, rel err ~7e-4 incl bf16):
  cc*cos(dhp) = a1p*a2p + b1*b2  ->  dHp^2 = 2(cc - D), sign from
  (a1p*b2 < a2p*b1); (cos,sin)(hbp) = normalize(C2p*(a1p,b1)+C1p*(a2p,b2))
  (short-arc bisector == CIEDE2000 hbp branch rules); T evaluated as a
  degree-4 polynomial in (cos hbp, sin hbp); dtheta Gaussian via
  delta^2 ~= w(1+w/12+w^2/90), w = 2(1-cos delta); sin(pi/3 eD) by odd
  poly; all sqrt/rsqrt/recip as Ln/Exp chains (single ACT table, no
  reloads); Rc shares the G chain (Cbar for both, +1.8e-5).

Engine placement measured on this HW (no DVE perf modes on this
toolchain; stt = 2 ALUs/instr; Pool ~2.4x cost/elem; ACT Square rides
free in the Ln/Exp table): squares on ACT, fused stt chains on DVE,
latency-tolerant side products on Pool. Chunks of [128, 512] x 6 planes,
8 chunks/core, emitted pairwise with a 40-op stagger so one chunk's
ACT-heavy phase overlaps the other's DVE-heavy phase; engine-clustered
topological order minimizes cross-engine semaphores (this walrus takes
1 sync wait/instr; extras split onto NoOps).

Measured: rel err 6.9e-4; HW exec ~493-501 us/iter (best-sampled
config: Pool rebalance of six latency-tolerant DVE products, NoOp
wait-splitting, 40-op chunk stagger; baseline same-methodology
607-649 us).
"""
import sys
import numpy as np

sys.path.insert(0, '/opt/trn_rl_repo')

import ml_dtypes

BF16NP = ml_dtypes.bfloat16
F32 = np.float32

_M = np.array([[0.412453, 0.357580, 0.180423],
               [0.212671, 0.715160, 0.072169],
               [0.019334, 0.119193, 0.950227]], dtype=np.float64)
_W = np.array([0.95047, 1.0, 1.08883], dtype=np.float64)
MW = (_M / _W[:, None]).astype(np.float32)
P25 = float(F32(25.0 ** 7))
LNP25 = float(F32(np.log(25.0 ** 7)))
_c30, _s30 = np.cos(np.pi/6), np.sin(np.pi/6)
_c6, _s6 = np.cos(np.deg2rad(6.)), np.sin(np.deg2rad(6.))
_c63, _s63 = np.cos(np.deg2rad(63.)), np.sin(np.deg2rad(63.))
GA0 = float(F32(1.0 - 0.24 - 0.20*_c63)); GA1 = float(F32(0.48 + 1.60*_c63))
GA2 = float(F32(-1.60*_c63))
AL0 = float(F32(-0.17*_c30 - 0.96*_c6)); AL1 = float(F32(1.28*_c6))
BE0 = float(F32(-0.17*_s30 + 0.32*_s6)); BE1 = float(F32(-1.28*_s6))
DE0 = float(F32(0.80*_s63)); DE1 = float(F32(-1.60*_s63))
C275 = float(F32(np.cos(np.deg2rad(275.)))); S275 = float(F32(np.sin(np.deg2rad(275.))))
KZ = float(F32((180.0/(25.0*np.pi))**2))
_m = (np.pi/3.0)**2
SP5 = float(F32(-_m**3/5040.)); SP3 = float(F32(_m**2/120.)); SP1 = float(F32(-_m/6.))
SRGB_LN_SCALE = float(F32(1/1.055)); SRGB_LN_BIAS = float(F32(0.055/1.055))
NEG2PI3 = float(F32(-2.0*np.pi/3.0))

# ---------------------------------------------------------------------------
# IR
#
# Tiles: name -> (width_in_F, dtype) ; dtype 'b' = bf16, 'f' = fp32.
# A tile of width w occupies w*F contiguous columns; ops may address a
# sub-slice (tile, lo, n) in F units.
# Op kinds:
#   ('act', func, dst, src, scale, bias [, accum])   ACT, dst/src tileslices
#   ('tt', eng, alu, dst, a, b)                      tensor_tensor eng 'v'/'p'
#   ('ts', dst, src, s1, s2, op0, op1)               DVE tensor_scalar
#   ('stt', dst, a, scalar, b, op0, op1)             DVE (a op0 s) op1 b
#   ('cp', eng, dst, src)                            tensor_copy
# tileslice: name or (name, lo, n)
# ---------------------------------------------------------------------------

def build_graph():
    """Layout r1 r2 g1 g2 b1 b2. Measured HW: no DVE perf modes; stt is the
    best DVE op (2 ALUs/1 instr); squares ride free on ACT (Square in table);
    Pool ~2.4x cost -> only latency-tolerant side ops. Rc shares the G chain
    (Cbar for both; +1.8e-5)."""
    tiles = {}
    ops = []

    def tile(name, w, dt):
        tiles[name] = (w, dt)
        return name

    A = ops.append
    tile('in6', 6, 'b')
    tile('ln6', 6, 'f')
    tile('lin6', 6, 'b')
    tile('xyz6', 6, 'b')
    tile('lnx6', 6, 'f')
    tile('f6', 6, 'b')
    tile('dxy', 2, 'b')
    tile('apair', 2, 'b')
    tile('bpair', 2, 'b')
    tile('dfy', 1, 'b')
    tile('sfy', 1, 'b')
    tile('sq1', 2, 'b')
    tile('sq2', 2, 'b')
    tile('spair', 2, 'b')
    tile('lns', 2, 'f')
    tile('Cp2', 2, 'b')
    tile('tG', 1, 'b')
    tile('uG', 1, 'f')
    tile('eG', 1, 'f')
    tile('vG', 1, 'f')
    tile('rG', 1, 'b')
    tile('opG2', 2, 'b')
    tile('abp', 2, 'b')
    tile('ssp', 2, 'b')
    tile('lnp', 2, 'f')
    tile('Cpp', 2, 'b')
    tile('Cbs', 1, 'b')
    tile('dCp', 1, 'b')
    tile('cc2', 1, 'b')
    tile('tu', 2, 'b')
    tile('t12', 1, 'b')
    tile('upq', 2, 'b')           # u | tPQ
    tile('luq', 2, 'f')
    tile('txy', 2, 'b')
    tile('msk', 1, 'b')
    tile('pqh', 2, 'b')
    tile('pqh2', 2, 'b')
    tile('pq', 2, 'b')
    tile('pqs', 2, 'b')
    tile('rPQ', 1, 'b')
    tile('cs', 2, 'b')
    tile('c2', 1, 'b')
    tile('gav', 1, 'b')
    tile('gaw', 1, 'b')
    tile('alv', 1, 'b')
    tile('dev', 1, 'b')
    tile('bev', 1, 'b')
    tile('qv', 1, 'b')
    tile('q2v', 1, 'b')
    tile('q3v', 1, 'b')
    tile('p1v', 1, 'b')
    tile('tS', 1, 'b')
    tile('Tv', 1, 'b')
    tile('ws', 1, 'b')
    tile('wv', 1, 'b')
    tile('da', 1, 'b')
    tile('db', 1, 'b')
    tile('d2', 1, 'b')
    tile('eD', 1, 'b')
    tile('yy', 1, 'b')
    tile('pa', 1, 'b')
    tile('pb', 1, 'b')
    tile('pd', 1, 'b')
    tile('sinv', 1, 'b')
    tile('Rt', 1, 'b')
    tile('mm', 1, 'b')
    tile('Rts', 1, 'b')
    tile('lsc', 1, 'f')
    tile('rSc', 1, 'b')
    tile('tC', 1, 'b')
    tile('l50', 1, 'b')
    tile('v20', 1, 'f')
    tile('rden', 1, 'b')
    tile('Sq', 1, 'b')
    tile('lsl', 1, 'f')
    tile('rSl', 1, 'b')
    tile('tL', 1, 'b')
    tile('qh', 1, 'b')
    tile('lsh', 1, 'f')
    tile('eH', 1, 'f')
    tile('tH', 1, 'b')
    tile('zb', 1, 'b')
    tile('za', 1, 'b')
    tile('z1', 1, 'b')
    tile('p1s', 1, 'b')
    tile('p2s', 1, 'b')
    tile('z2', 1, 'b')
    tile('zs', 1, 'b')
    tile('zc', 1, 'b')
    tile('lz', 1, 'f')
    tile('dE', 1, 'f')

    S = lambda t, lo, n=1: (t, lo, n)

    # stage A
    A(('act', 'Ln', 'ln6', 'in6', SRGB_LN_SCALE, SRGB_LN_BIAS))
    A(('act', 'Exp', 'lin6', 'ln6', 2.4, 0.0))
    # XYZ: per row k: u = (r*alpha_k) + g ; X' = (b*gamma_k) + u ; scale
    # m01_k folded via explicit ts on 2-wide (row-normalized form needs a
    # final scale; use stt chain with true coefficients instead: 3 stt)
    for k in range(3):
        dst = S('xyz6', 2*k, 2)
        r, g, b = S('lin6', 0, 2), S('lin6', 2, 2), S('lin6', 4, 2)
        A(('ts', dst, r, float(MW[k, 0]), None, 'mult', None))
        A(('stt', dst, g, float(MW[k, 1]), dst, 'mult', 'add'))
        A(('stt', dst, b, float(MW[k, 2]), dst, 'mult', 'add'))
    A(('act', 'Ln', 'lnx6', 'xyz6', 1.0, 0.0))
    A(('act', 'Exp', 'f6', 'lnx6', 1.0/3.0, 0.0))
    fxp, fyp, fzp = S('f6', 0, 2), S('f6', 2, 2), S('f6', 4, 2)
    A(('tt', 'v', 'subtract', 'dxy', fxp, fyp))
    A(('tt', 'v', 'subtract', 'bpair', fyp, fzp))
    A(('tt', 'p', 'subtract', 'dfy', S('f6', 3), S('f6', 2)))
    A(('tt', 'p', 'add', 'sfy', S('f6', 2), S('f6', 3)))
    A(('ts', 'apair', 'dxy', 500.0, None, 'mult', None))
    A(('ts', 'bpair', 'bpair', 200.0, None, 'mult', None))
    # s pair (squares on ACT)
    A(('act', 'Square', 'sq1', 'apair', 1.0, 0.0))
    A(('act', 'Square', 'sq2', 'bpair', 1.0, 0.0))
    A(('tt', 'v', 'add', 'spair', 'sq1', 'sq2'))
    A(('act', 'Ln', 'lns', 'spair', 1.0, 0.0))
    A(('act', 'Exp', 'Cp2', 'lns', 0.5, 0.0))
    # G chain (shared with Rc: Rc = 2*rG)
    A(('tt', 'p', 'add', 'tG', S('Cp2', 0), S('Cp2', 1)))
    A(('act', 'Ln', 'uG', 'tG', 0.5, 0.0))
    A(('act', 'Exp', 'eG', 'uG', -7.0, LNP25))
    A(('act', 'Ln', 'vG', 'eG', 1.0, 1.0))
    A(('act', 'Exp', 'rG', 'vG', -0.5, 0.0))
    A(('ts', S('opG2', 0), 'rG', -0.5, 1.5, 'mult', 'add'))
    A(('cp', 'v', S('opG2', 1), S('opG2', 0)))
    A(('tt', 'v', 'mult', 'abp', 'apair', 'opG2'))
    A(('act', 'Square', 'sq1', 'abp', 1.0, 0.0))
    A(('tt', 'v', 'add', 'ssp', 'sq1', 'sq2'))
    A(('act', 'Ln', 'lnp', 'ssp', 1.0, 0.0))
    A(('act', 'Exp', 'Cpp', 'lnp', 0.5, 0.0))
    A(('tt', 'p', 'add', 'Cbs', S('Cpp', 0), S('Cpp', 1)))
    A(('tt', 'v', 'subtract', 'dCp', S('Cpp', 1), S('Cpp', 0)))
    # u = 2cc - 2(t1+t2); cc2 = 2*C1p*C2p
    A(('stt', 'cc2', S('Cpp', 0), 2.0, S('Cpp', 1), 'mult', 'mult'))
    A(('tt', 'p', 'mult', S('tu', 0), S('abp', 0), S('abp', 1)))
    A(('tt', 'p', 'mult', S('tu', 1), S('bpair', 0), S('bpair', 1)))
    A(('tt', 'p', 'add', 't12', S('tu', 0), S('tu', 1)))
    A(('stt', S('upq', 0), 't12', -2.0, 'cc2', 'mult', 'add'))
    A(('ts', S('upq', 0), S('upq', 0), 0.0, None, 'max', None))
    # sign
    A(('tt', 'p', 'mult', S('txy', 0), S('abp', 0), S('bpair', 1)))
    A(('tt', 'p', 'mult', S('txy', 1), S('abp', 1), S('bpair', 0)))
    A(('tt', 'v', 'is_lt', 'msk', S('txy', 0), S('txy', 1)))
    # hbp direction
    A(('tt', 'p', 'mult', S('pqh', 0), S('Cpp', 1), S('abp', 0)))
    A(('tt', 'p', 'mult', S('pqh', 1), S('Cpp', 1), S('bpair', 0)))
    A(('tt', 'p', 'mult', S('pqh2', 0), S('Cpp', 0), S('abp', 1)))
    A(('tt', 'p', 'mult', S('pqh2', 1), S('Cpp', 0), S('bpair', 1)))
    A(('tt', 'v', 'add', 'pq', 'pqh', 'pqh2'))
    A(('act', 'Square', 'pqs', 'pq', 1.0, 0.0))
    A(('tt', 'v', 'add', S('upq', 1), S('pqs', 0), S('pqs', 1)))
    A(('act', 'Ln', 'luq', 'upq', 1.0, 0.0))
    A(('act', 'Exp', 'rPQ', S('luq', 1), -0.5, 0.0))
    A(('tt', 'v', 'mult', S('cs', 0), S('pq', 0), 'rPQ'))
    A(('tt', 'v', 'mult', S('cs', 1), S('pq', 1), 'rPQ'))
    # T = gam(c2) + c*al(c2) + s*(be(c2) + c*de(c2)) ; c2 = c^2
    A(('act', 'Square', 'c2', S('cs', 0), 1.0, 0.0))
    A(('ts', 'gav', 'c2', GA2, GA1, 'mult', 'add'))
    A(('tt', 'v', 'mult', 'gaw', 'gav', 'c2'))
    A(('ts', 'alv', 'c2', AL1, AL0, 'mult', 'add'))
    A(('ts', 'dev', 'c2', DE1, DE0, 'mult', 'add'))
    A(('ts', 'bev', 'c2', BE1, BE0, 'mult', 'add'))
    A(('tt', 'p', 'mult', 'qv', 'dev', S('cs', 0)))
    A(('tt', 'p', 'add', 'q2v', 'bev', 'qv'))
    A(('tt', 'p', 'mult', 'q3v', 'q2v', S('cs', 1)))
    A(('tt', 'p', 'mult', 'p1v', 'alv', S('cs', 0)))
    A(('stt', 'tS', 'gaw', GA0, 'p1v', 'add', 'add'))
    A(('tt', 'v', 'add', 'Tv', 'tS', 'q3v'))
    # zsq / dtheta
    A(('ts', 'ws', S('cs', 1), -2.0*S275, 2.0, 'mult', 'add'))
    A(('stt', 'wv', S('cs', 0), -2.0*C275, 'ws', 'mult', 'add'))
    A(('ts', 'da', 'wv', 1.0/90.0, 1.0/12.0, 'mult', 'add'))
    A(('tt', 'p', 'mult', 'db', 'da', 'wv'))
    A(('stt', 'd2', 'db', 1.0, 'wv', 'add', 'mult'))
    A(('act', 'Exp', 'eD', 'd2', -KZ, 0.0))
    # sinv = eD*(1 + y*((SP5*y+SP3)*y+SP1)), y = eD^2
    A(('act', 'Square', 'yy', 'eD', 1.0, 0.0))
    A(('ts', 'pa', 'yy', SP5, SP3, 'mult', 'add'))
    A(('tt', 'p', 'mult', 'pb', 'pa', 'yy'))
    A(('stt', 'pd', 'pb', SP1, 'yy', 'add', 'mult'))
    A(('stt', 'sinv', 'pd', 1.0, 'eD', 'add', 'mult'))
    # Rt = K*sinv*Rc, Rc = 2*rG (K & 2 & -2pi/3 folded into zb const)
    A(('tt', 'p', 'mult', 'Rt', 'sinv', 'rG'))
    A(('ts', 'mm', 'msk', -2.0, 1.0, 'mult', 'add'))
    A(('tt', 'p', 'mult', 'Rts', 'Rt', 'mm'))
    # Sc recip
    A(('act', 'Ln', 'lsc', 'Cbs', 0.0225, 1.0))
    A(('act', 'Exp', 'rSc', 'lsc', -1.0, 0.0))
    A(('tt', 'v', 'mult', 'tC', 'dCp', 'rSc'))
    # Sl
    A(('act', 'Square', 'l50', 'sfy', 58.0, -66.0))
    A(('act', 'Ln', 'v20', 'l50', 1.0, 20.0))
    A(('act', 'Exp', 'rden', 'v20', -0.5, 0.0))
    A(('stt', 'Sq', 'l50', 0.015, 'rden', 'mult', 'mult'))
    A(('act', 'Ln', 'lsl', 'Sq', 1.0, 1.0))
    A(('act', 'Exp', 'rSl', 'lsl', -1.0, 0.0))
    A(('stt', 'tL', 'dfy', 116.0, 'rSl', 'mult', 'mult'))
    # Sh & tH
    A(('tt', 'p', 'mult', 'qh', 'Tv', 'Cbs'))
    A(('act', 'Ln', 'lsh', 'qh', 0.0075, 1.0))
    A(('stt', 'eH', S('luq', 0), 0.5, 'lsh', 'mult', 'subtract'))
    A(('act', 'Exp', 'tH', 'eH', 1.0, 0.0))
    # z ; NEG4PI3 = 2*(-2pi/3) for Rc=2rG fold
    A(('stt', 'zb', 'Rts', NEG2PI3, 'tH', 'mult', 'mult'))
    A(('tt', 'v', 'add', 'za', 'zb', 'tC'))
    A(('tt', 'v', 'mult', 'z1', 'za', 'tC'))
    A(('act', 'Square', 'p1s', 'tL', 1.0, 0.0))
    A(('act', 'Square', 'p2s', 'tH', 1.0, 0.0))
    A(('tt', 'p', 'add', 'z2', 'p1s', 'p2s'))
    A(('tt', 'v', 'add', 'zs', 'z1', 'z2'))
    A(('ts', 'zc', 'zs', 0.0, None, 'max', None))
    A(('act', 'Ln', 'lz', 'zc', 1.0, 0.0))
    A(('act', 'Exp', 'dE', 'lz', 0.5, 0.0, 'acc'))
    return tiles, ops


# ---------------------------------------------------------------------------
# numpy executor (dtype-emulating)
# ---------------------------------------------------------------------------

_ALU_NP = {
    'mult': lambda a, b: a*b, 'add': lambda a, b: a+b,
    'subtract': lambda a, b: a-b, 'max': np.maximum,
    'is_lt': lambda a, b: (a < b).astype(np.float32),
}


def run_graph_np(tiles, ops, in6):
    """in6: [N,6] fp32 (r1 g1 b1 r2 g2 b2 already bf16-rounded upstream).
    Returns dE [N]."""
    N = in6.shape[0]
    buf = {}
    for name, (w, dt) in tiles.items():
        buf[name] = np.zeros((N, w), np.float32)
    buf['in6'][:] = in6

    def rd(ts_):
        name, lo, n = (ts_, 0, tiles[ts_][0]) if isinstance(ts_, str) else ts_
        return buf[name][:, lo:lo+n]

    def wr(ts_, val):
        name, lo, n = (ts_, 0, tiles[ts_][0]) if isinstance(ts_, str) else ts_
        dt = tiles[name][1]
        v = np.asarray(val, np.float32)
        if dt == 'b':
            v = v.astype(BF16NP).astype(np.float32)
        buf[name][:, lo:lo+n] = v

    f = lambda x: np.asarray(x, np.float32)
    with np.errstate(divide='ignore', invalid='ignore', over='ignore'):
        for op in ops:
            k = op[0]
            if k == 'act':
                func, dst, src, scale, bias = op[1], op[2], op[3], op[4], op[5]
                x = f(rd(src)*F32(scale) + F32(bias))
                if func == 'Ln':
                    v = np.log(x, dtype=np.float32)
                elif func == 'Exp':
                    v = np.exp(x, dtype=np.float32)
                elif func == 'Square':
                    v = f(x*x)
                else:
                    raise ValueError(func)
                wr(dst, v)
            elif k == 'tt':
                _, eng, alu, dst, a, b = op
                wr(dst, _ALU_NP[alu](rd(a), rd(b)))
            elif k == 'ts':
                _, dst, src, s1, s2, op0, op1 = op
                v = f(_ALU_NP[op0](rd(src), F32(s1)))
                if op1 is not None:
                    v = f(_ALU_NP[op1](v, F32(s2)))
                wr(dst, v)
            elif k == 'stt':
                _, dst, a, scalar, b, op0, op1 = op
                v = f(_ALU_NP[op0](rd(a), F32(scalar)))
                wr(dst, f(_ALU_NP[op1](v, rd(b))))
            elif k == 'cp':
                _, eng, dst, src = op
                wr(dst, rd(src))
            else:
                raise ValueError(k)
    dE = buf['dE'][:, 0]
    return np.where(np.isnan(dE), 0.0, dE).astype(np.float32)


def test_graph():
    sys.path.insert(0, '/root/problem')
    import reference as ref
    inputs = ref.setup_inputs()
    expected = float(ref.reference(**inputs))
    img1 = np.asarray(inputs['img1'])
    img2 = np.asarray(inputs['img2'])
    # in6 layout: [N, 6]
    N = img1.shape[0]*img1.shape[2]*img1.shape[3]
    in6 = np.empty((N, 6), np.float32)
    for k in range(3):
        in6[:, 2*k] = img1[:, k].ravel()
        in6[:, 2*k+1] = img2[:, k].ravel()
    in6 = in6.astype(BF16NP).astype(np.float32)   # host bf16 conversion
    tiles, ops = build_graph()
    # chunked to bound memory
    tot = 0.0
    CH = 1 << 20
    for i in range(0, N, CH):
        dE = run_graph_np(tiles, ops, in6[i:i+CH])
        tot += dE.astype(np.float64).sum()
    mean = tot / N
    rel = (mean-expected)/expected
    print(f"graph sim: mean={mean:.6f} expected={expected:.6f} rel={rel:+.3e}")
    n_act = sum(1 for o in ops if o[0] == 'act')
    fops = sum((tiles[o[2] if isinstance(o[2], str) else o[2][0]][0]
                if isinstance(o[2], str) or True else 0) for o in ops if o[0] == 'act')
    # count ACT F-ops precisely
    fops = 0
    for o in ops:
        if o[0] == 'act':
            d = o[2]
            name, lo, n = (d, 0, tiles[d][0]) if isinstance(d, str) else d
            fops += n
    nv = sum(1 for o in ops if o[0] in ('ts', 'stt') or (o[0] == 'tt' and o[1] == 'v') or (o[0] == 'cp' and o[1] == 'v'))
    npool = sum(1 for o in ops if o[0] == 'tt' and o[1] == 'p')
    print(f"ops: {len(ops)} total, ACT instr {n_act} ({fops} F-ops), DVE {nv}, Pool {npool}")


# ---------------------------------------------------------------------------
# Bass emission
# ---------------------------------------------------------------------------

def _collect_act_biases(ops):
    vals = set()
    for op in ops:
        if op[0] == 'act':
            b = float(F32(op[5]))
            if b != 0.0:
                vals.add(b)
    return sorted(vals)


def _slice_of(tiles, ts_):
    if isinstance(ts_, str):
        return ts_, 0, tiles[ts_][0]
    return ts_


def build_bass(repeats=1, dyn=False, FD=512, nch=8, ninf=2, stagger=40):
    """ninf: chunks in flight (tag namespaces)."""
    import concourse.bass as bass
    import concourse.mybir as mybir
    from concourse import tile
    from concourse import tile_utils as _tu

    AF = mybir.ActivationFunctionType
    ALU = mybir.AluOpType
    DT = mybir.dt.float32
    BT = mybir.dt.bfloat16
    P = 128

    tiles, ops = build_graph()

    # liveness: last read index per tile name (whole-tile granularity)
    def op_reads(op):
        k = op[0]
        if k == 'act':
            return [op[3]]
        if k == 'tt':
            return [op[4], op[5]]
        if k == 'ts':
            return [op[2]]
        if k == 'stt':
            return [op[2], op[4]]
        if k == 'cp':
            return [op[3]]
        raise ValueError(k)

    def op_writes(op):
        k = op[0]
        if k == 'act':
            return op[2]
        if k == 'tt':
            return op[3]
        if k in ('ts', 'stt'):
            return op[1]
        if k == 'cp':
            return op[2]
        raise ValueError(k)

    def op_engine(op):
        k = op[0]
        if k == 'act':
            return 'A'
        if k == 'tt' and op[1] == 'p':
            return 'P'
        if k == 'cp' and op[1] == 'p':
            return 'P'
        return 'V'

    def cluster_schedule(ops):
        n = len(ops)
        writes = {}
        preds = [set() for _ in range(n)]
        for i, op in enumerate(ops):
            for r in op_reads(op):
                nm = r if isinstance(r, str) else r[0]
                if nm in writes:
                    preds[i].add(writes[nm])
            w = op_writes(op)
            wnm = w if isinstance(w, str) else w[0]
            # WAW/WAR: depend on previous writer and previous readers
            if wnm in writes:
                preds[i].add(writes[wnm])
            writes[wnm] = i
        # also WAR: a write must come after earlier reads of same tile
        readers = {}
        writes2 = {}
        for i, op in enumerate(ops):
            w = op_writes(op)
            wnm = w if isinstance(w, str) else w[0]
            if wnm in readers:
                preds[i] |= readers[wnm]
            for r in op_reads(op):
                nm = r if isinstance(r, str) else r[0]
                readers.setdefault(nm, set()).add(i)
        npred = [len(p) for p in preds]
        succs = [[] for _ in range(n)]
        for i, p in enumerate(preds):
            for j in p:
                succs[j].append(i)
        import heapq
        ready = {e: [] for e in 'APV'}
        for i in range(n):
            if npred[i] == 0:
                heapq.heappush(ready[op_engine(ops[i])], i)
        out = []
        cur = 'A'
        while len(out) < n:
            if not ready[cur]:
                cands = [e for e in 'APV' if ready[e]]
                cur = min(cands, key=lambda e: ready[e][0])
            i = heapq.heappop(ready[cur])
            out.append(ops[i])
            for j in succs[i]:
                npred[j] -= 1
                if npred[j] == 0:
                    heapq.heappush(ready[op_engine(ops[j])], j)
        return out

    ops = cluster_schedule(ops)

    last_use = {}
    for i, op in enumerate(ops):
        for r in op_reads(op):
            nm = r if isinstance(r, str) else r[0]
            last_use[nm] = i
        wnm = op_writes(op)
        wnm = wnm if isinstance(wnm, str) else wnm[0]
        last_use.setdefault(wnm, i)

    if getattr(_tu, 'max_sbuf_usage', 0) < 204 * 1024:
        _tu.max_sbuf_usage = 204 * 1024
    nc = bass.Bass()

    for v in _collect_act_biases(ops):
        if (DT, v) in nc.const_aps.aps:
            continue
        t = nc.alloc_sbuf_tensor(f"const-f32-{v}", [P, 1], DT)
        nc.gpsimd.memset(t.ap(), v)
        nc.const_aps.aps[(DT, v)] = t.ap()
    nc.all_engine_barrier()

    in_cols = nch * 6 * FD
    x_ext = nc.dram_tensor("x", [P, in_cols], BT, kind="ExternalInput")
    acc_ext = nc.dram_tensor("acc", [P, nch], DT, kind="ExternalOutput")

    alu = lambda n: getattr(ALU, n)

    with tile.TileContext(nc) as tc:
        with tc.tile_pool(name="io", bufs=2) as iop, \
             tc.tile_pool(name="wk", bufs=1) as wk:
            acc_t = wk.tile([P, nch], DT, tag="acc", name="acc")

            import contextlib
            rep_ctx = tc.For_i(0, repeats, 1) if dyn else None

            class Chunk:
                def __init__(self, ci):
                    self.ci = ci
                    pi = ci % ninf
                    self.pi = pi
                    self.t_in = iop.tile([P, 6*FD], BT, tag=f"in_{pi}",
                                         name=f"in_{ci}")
                    nc.sync.dma_start(self.t_in[:],
                                      x_ext[:, ci*6*FD:(ci+1)*6*FD])
                    self.bound = {'in6': self.t_in}
                    self.free = {('f', 1): [], ('b', 1): [], ('f', 2): [],
                                 ('b', 2): [], ('b', 4): [], ('f', 6): [],
                                 ('b', 6): []}
                    self.nslot = {}

                def tile_of(self, nm):
                    if nm in self.bound:
                        return self.bound[nm]
                    w, dt = tiles[nm]
                    key = (dt, w)
                    fl = self.free.setdefault(key, [])
                    if fl:
                        t = fl.pop(0)
                    else:
                        idx = self.nslot.get(key, 0)
                        self.nslot[key] = idx + 1
                        tag = f"s_{dt}{w}_{idx}_{self.pi}"
                        t = wk.tile([P, w*FD], BT if dt == 'b' else DT,
                                    tag=tag, name=f"{tag}_c{self.ci}")
                    self.bound[nm] = t
                    return t

                def ap(self, ts_):
                    nm, lo, n = _slice_of(tiles, ts_)
                    t = self.tile_of(nm)
                    return t[:, lo*FD:(lo+n)*FD]

                def release(self, i, op):
                    for r in op_reads(op):
                        nm = r if isinstance(r, str) else r[0]
                        if nm in self.bound and last_use.get(nm) == i \
                                and nm != 'in6':
                            w, dt = tiles[nm]
                            self.free.setdefault((dt, w), []).append(
                                self.bound.pop(nm))

                def emit(self, i):
                    op = ops[i]
                    k = op[0]
                    if k == 'act':
                        func, dst, src, scale, bias = op[1:6]
                        accum = len(op) > 6
                        kw = {}
                        if accum:
                            kw['accum_out'] = acc_t[:, self.ci:self.ci+1]
                        nc.scalar.activation(self.ap(dst), self.ap(src),
                                             getattr(AF, func),
                                             bias=float(F32(bias)),
                                             scale=float(F32(scale)), **kw)
                    elif k == 'tt':
                        _, eng, aluop, dst, a, b = op
                        e = nc.vector if eng == 'v' else nc.gpsimd
                        e.tensor_tensor(self.ap(dst), self.ap(a), self.ap(b),
                                        alu(aluop))
                    elif k == 'ts':
                        _, dst, src, s1, s2, op0, op1 = op
                        if op1 is None:
                            nc.vector.tensor_scalar(
                                self.ap(dst), self.ap(src), float(F32(s1)),
                                None, alu(op0))
                        else:
                            nc.vector.tensor_scalar(
                                self.ap(dst), self.ap(src), float(F32(s1)),
                                float(F32(s2)), alu(op0), alu(op1))
                    elif k == 'stt':
                        _, dst, a, scalar, b, op0, op1 = op
                        nc.vector.scalar_tensor_tensor(
                            self.ap(dst), self.ap(a), float(F32(scalar)),
                            self.ap(b), alu(op0), alu(op1))
                    elif k == 'cp':
                        _, eng, dst, src = op
                        e = nc.vector if eng == 'v' else nc.gpsimd
                        e.tensor_copy(self.ap(dst), self.ap(src))
                    else:
                        raise ValueError(k)
                    self.release(i, op)

            with (rep_ctx if dyn else contextlib.nullcontext()):
                n = len(ops)
                for grp in range(nch // ninf):
                    chunks = [Chunk(grp*ninf + j) for j in range(ninf)]
                    for i in range(n + stagger*(ninf-1)):
                        for j, c in enumerate(chunks):
                            k = i - stagger*j
                            if 0 <= k < n:
                                c.emit(k)

            nc.scalar.dma_start(acc_ext[:], acc_t[:])

    _split_sync_waits(nc)
    return nc


def _split_sync_waits(nc, max_waits=1):
    """This walrus rejects >1 sync wait per instruction; keep the first wait
    on the instruction and move extras onto same-engine NoOps inserted right
    before (sequencers issue in order)."""
    import concourse.mybir as mybir
    n = [0]
    for fn in nc.m.functions:
        for bb in fn.blocks:
            insts = bb.instructions
            out = []
            changed = False
            for inst in insts:
                si = getattr(inst, "sync_info", None)
                waits = list(si.on_wait) if (si and si.on_wait) else []
                if len(waits) > max_waits:
                    keep = waits[:max_waits]
                    for w in waits[max_waits:]:
                        n[0] += 1
                        nop = mybir.InstNoOp(name=f"I-wsplit-{n[0]}", ins=[],
                                             outs=[])
                        nop.engine = inst.engine
                        nop.sync_info = mybir.SyncInfo(on_wait=[w],
                                                       on_update=[])
                        out.append(nop)
                    inst.sync_info = mybir.SyncInfo(
                        on_wait=keep, on_update=list(si.on_update or []))
                    changed = True
                out.append(inst)
            if changed:
                del insts[:]
                insts.extend(out)


# ---------------------------------------------------------------------------
# host entry
# ---------------------------------------------------------------------------

_CACHED = {}
FD = 512


def _prearrange(sh1, sh2):
    """sh1, sh2: [2,3,512,512] fp32 -> [128, 8*6*FD] bf16.
    chunk (b, quarter): planes r1 g1 b1 r2 g2 b2, each [128, FD]."""
    nq = 2048 // FD
    out = np.empty((128, 2*nq*6*FD), dtype=BF16NP)
    x1 = sh1.reshape(2, 3, 128, nq, FD)
    x2 = sh2.reshape(2, 3, 128, nq, FD)
    for b in range(2):
        for h in range(nq):
            ci = b*nq + h
            base = ci*6*FD
            for k in range(3):
                out[:, base+2*k*FD:base+(2*k+1)*FD] = x1[b, k, :, h]
                out[:, base+(2*k+1)*FD:base+(2*k+2)*FD] = x2[b, k, :, h]
    return out


def kernel(img1, img2):
    from concourse.bass_utils import run_bass_kernel_spmd

    img1 = np.asarray(img1)
    img2 = np.asarray(img2)
    n_cores = 8
    per = img1.shape[0] // n_cores

    if 'nc' not in _CACHED:
        _CACHED['nc'] = build_bass()
    nc = _CACHED['nc']

    in_maps = []
    for c in range(n_cores):
        s = slice(c*per, (c+1)*per)
        in_maps.append({"x": _prearrange(img1[s], img2[s])})

    res = run_bass_kernel_spmd(nc, in_maps, list(range(n_cores)))
    total = 0.0
    for r in res.results:
        total += r["acc"].astype(np.float64).sum()
    mean = total / (img1.shape[0] * img1.shape[2] * img1.shape[3])
    return np.float32(mean)


if __name__ == '__main__':
    test_graph()
